# revision 2
# baseline (speedup 1.0000x reference)
"""Trainium2 Bass kernel: DGCNN Zernike-monomial interwiner (nn_DGCNN_8839042695322).

Computes, per point p=(x,y,z):
  out[.., 16, 4] = concat_l( einsum(zernike_monoms(p)[l], Wl) ) for l=0..3
Every output channel is a degree<=3 polynomial in (x,y,z); weights are folded
host-side into per-channel scalar immediates.

Memory-bound problem: the f32 output is 32 MB/core. The device computes and
stores the output in bf16 (norm rel err ~7e-3, well under the 2e-2 gate),
halving HBM write traffic. The SBUF output tile is CHANNEL-major [P, 64, T]
so every compute op writes unit-stride runs (the interleaved [P, T, 64]
layout costs 2-3x on DVE/ACT); the DRAM layout is tile-blocked accordingly
and the host does the final (T, 64) transpose during unsharding.

x is pre-transposed on host to [P, 3, COLS] bf16 so coordinate planes are
contiguous SBUF views (no on-device deinterleave) and all-bf16 operands
enable the DVE 2x perf mode.

Sharding: pure data parallel over the batch axis across 8 NeuronCores.
"""

import numpy as np
import ml_dtypes

import concourse.bacc as bacc
import concourse.tile as tile
from concourse import mybir
from concourse.bass_utils import run_bass_kernel_spmd

# Problem geometry (hardcoded per spec: x [32, 32768, 3] f32, 8 cores).
B, N, M_CORES = 32, 32768, 8
PTS_PER_CORE = B * N // M_CORES  # 131072
P = 128                          # SBUF partitions
COLS = PTS_PER_CORE // P         # 1024 points per partition
# Graduated schedule: tiny first tile issues the first output DMA early
# (fill phase), large tiles amortize per-op fixed costs in steady state.
TILE_LENS = [64, 192, 384, 384]
assert sum(TILE_LENS) == COLS
# Channel groups (m = spherical index 0..15) whose 4 channel-writes run on
# the Scalar (ACT) engine; the rest run on Vector (DVE). Balanced so both
# engines stay under the per-tile DMA time.
ACT_GROUPS = (4, 5, 6, 7, 10, 14)

# Real spherical-harmonic constants (match reference).
C0 = 0.28209479177387814
C1 = 0.4886025119029199
C2_XY = 1.0925484305920792
C2_0 = 0.31539156525252005
C2_2 = 0.5462742152960396
C3_3 = 0.5900435899266435
C3_2 = 2.890611442640554
C3_1 = 0.4570457994644658
C3_0 = 0.3731763325901154
C3_P2 = 1.445305721320277

_cache: dict = {}


def _host_constants(W0, b0, W1, W2, W3):
    """Fold interwiner weights into per-channel scalars.

    Returns dict with:
      A0, B0   [4]: l0 channel u = A0[u] + B0[u]*n2
      AA1, BB1 [4]: s'_u = AA1[u] + BB1[u]*n2; l1 channel (m,u) = p_m * s'_u
      w2 [5,4], w3 [7,4]: channel (m,u) = base_m * w[m,u]
    """
    coef2 = np.array([C2_XY, C2_XY, C2_0, C2_XY, C2_2], dtype=np.float64)
    # base for m12 is d3 = z2 - 0.6*n2 = (2z2-3x2-3y2)/5, so fold the 5 in.
    coef3 = np.array(
        [C3_3, C3_2, C3_1, 5.0 * C3_0, C3_1, C3_P2, C3_3], dtype=np.float64
    )
    w2 = (coef2[:, None] * W2[0][None, :].astype(np.float64)).astype(np.float32)
    w3 = (coef3[:, None] * W3[0][None, :].astype(np.float64)).astype(np.float32)
    A0 = (C0 * W0[0].astype(np.float64) + b0.astype(np.float64)).astype(np.float32)
    B0 = (C0 * W0[1].astype(np.float64)).astype(np.float32)
    AA1 = (C1 * W1[0].astype(np.float64)).astype(np.float32)
    BB1 = (C1 * W1[1].astype(np.float64)).astype(np.float32)
    return dict(A0=A0, B0=B0, AA1=AA1, BB1=BB1, w2=w2, w3=w3)


def _build_program(consts, tile_lens, act_groups):
    tile_lens = list(tile_lens)
    dt = mybir.dt.bfloat16
    F = mybir.ActivationFunctionType
    ALU = mybir.AluOpType
    A0, B0 = consts["A0"], consts["B0"]
    AA1, BB1 = consts["AA1"], consts["BB1"]
    w2, w3 = consts["w2"], consts["w3"]

    nc = bacc.Bacc(
        "TRN2", target_bir_lowering=False, debug=False, num_devices=M_CORES
    )
    xin = nc.dram_tensor("xin", [P, 3, COLS], dt, kind="ExternalInput").ap()
    yout = nc.dram_tensor("yout", [P, COLS * 64], dt, kind="ExternalOutput").ap()

    with tile.TileContext(nc) as tc:
        with (
            tc.tile_pool(name="xpool", bufs=1) as xpool,
            tc.tile_pool(name="bases", bufs=2) as bases_pool,
            tc.tile_pool(name="opool", bufs=2) as opool,
        ):
            # whole input resident: 6KB/partition. Split the load so the
            # first tile's columns land fast.
            xall = xpool.tile([P, 3, COLS], dt, name="xall")
            t0 = tile_lens[0]
            nc.sync.dma_start(out=xall[:, :, 0:t0], in_=xin[:, :, 0:t0])
            nc.sync.dma_start(out=xall[:, :, t0:], in_=xin[:, :, t0:])

            ts = 0
            for it, T in enumerate(tile_lens):
                px = xall[:, 0, ts : ts + T]
                py = xall[:, 1, ts : ts + T]
                pz = xall[:, 2, ts : ts + T]

                def plane(tag):
                    return bases_pool.tile([P, T], dt, name=tag)

                xy = plane("xy")
                yz = plane("yz")
                xz = plane("xz")
                x2 = plane("x2")
                y2 = plane("y2")
                z2 = plane("z2")
                n2a = plane("n2a")
                n2 = plane("n2")
                x2my2 = plane("x2my2")
                t2a = plane("t2a")
                a3 = plane("a3")
                b3 = plane("b3")
                c3 = plane("c3")
                d3 = plane("d3")
                xyz = plane("xyz")
                zxmy = plane("zxmy")
                xb = plane("xb")
                ya = plane("ya")
                yc = plane("yc")
                zd = plane("zd")
                xc = plane("xc")
                sp = bases_pool.tile([P, 4, T], dt, name="sp")

                ov = opool.tile([P, 64, T], dt, name="ov")

                # --- DVE planes; xy/yz/xz first so ACT groups unblock early
                nc.vector.tensor_mul(xy, px, py)
                nc.vector.tensor_mul(yz, py, pz)
                nc.vector.tensor_mul(xz, px, pz)
                nc.vector.tensor_mul(x2, px, px)
                nc.vector.tensor_mul(y2, py, py)
                nc.vector.tensor_mul(z2, pz, pz)
                nc.vector.tensor_add(n2a, x2, y2)
                nc.vector.tensor_add(n2, n2a, z2)
                nc.vector.tensor_mul(xyz, xy, pz)
                nc.vector.tensor_sub(x2my2, x2, y2)
                # t2a = 3*z2 - n2 = 2z2 - x2 - y2
                nc.vector.scalar_tensor_tensor(
                    t2a, z2, 3.0, n2, op0=ALU.mult, op1=ALU.subtract
                )
                nc.vector.tensor_mul(zxmy, pz, x2my2)
                # a3 = 3*x2 - y2
                nc.vector.scalar_tensor_tensor(
                    a3, x2, 3.0, y2, op0=ALU.mult, op1=ALU.subtract
                )
                # b3 = x2 - 3*y2
                nc.vector.scalar_tensor_tensor(
                    b3, y2, -3.0, x2, op0=ALU.mult, op1=ALU.add
                )
                # c3 = 5*z2 - n2 = 4z2 - x2 - y2
                nc.vector.scalar_tensor_tensor(
                    c3, z2, 5.0, n2, op0=ALU.mult, op1=ALU.subtract
                )
                # d3 = z2 - 0.6*n2 (= (2z2-3x2-3y2)/5)
                nc.vector.scalar_tensor_tensor(
                    d3, n2, -0.6, z2, op0=ALU.mult, op1=ALU.add
                )
                nc.vector.tensor_mul(xb, px, b3)
                nc.vector.tensor_mul(ya, py, a3)
                nc.vector.tensor_mul(yc, py, c3)
                nc.vector.tensor_mul(zd, pz, d3)
                nc.vector.tensor_mul(xc, px, c3)

                # s'_u = AA1[u] + BB1[u]*n2 (DVE TS 2-imm)
                for u in range(4):
                    nc.vector.tensor_scalar(
                        sp[:, u, :], n2, float(BB1[u]), float(AA1[u]),
                        op0=ALU.mult, op1=ALU.add,
                    )

                # l=0 (ch 0..3): out = A0[u] + B0[u]*n2
                for u in range(4):
                    nc.vector.tensor_scalar(
                        ov[:, u, :], n2, float(B0[u]), float(A0[u]),
                        op0=ALU.mult, op1=ALU.add,
                    )

                # l=1 (ch 4..15): out[m,u] = p_m * s'_u (order y,z,x),
                # one broadcast op per m.
                for mi, pm in enumerate((py, pz, px)):
                    nc.vector.tensor_mul(
                        ov[:, 4 + 4 * mi : 8 + 4 * mi, :],
                        pm.unsqueeze(1).broadcast_to((P, 4, T)),
                        sp,
                    )

                # simple groups (ch 16..63): out[m,u] = base_m * w[m,u]
                groups = {
                    4: (xy, w2[0]), 5: (yz, w2[1]), 6: (t2a, w2[2]),
                    7: (xz, w2[3]), 8: (x2my2, w2[4]),
                    9: (ya, w3[0]), 10: (xyz, w3[1]), 11: (yc, w3[2]),
                    12: (zd, w3[3]), 13: (xc, w3[4]), 14: (zxmy, w3[5]),
                    15: (xb, w3[6]),
                }
                # ACT first in plane-availability order
                act_order = [m for m in (4, 5, 7, 10, 6, 14, 8, 9, 11, 12, 13, 15)
                             if m in act_groups]
                for m in act_order:
                    base, wrow = groups[m]
                    for u in range(4):
                        nc.scalar.activation(
                            ov[:, 4 * m + u, :], base, F.Copy,
                            scale=float(wrow[u]),
                        )
                for m in sorted(set(groups) - set(act_groups)):
                    base, wrow = groups[m]
                    for u in range(4):
                        nc.vector.tensor_scalar(
                            ov[:, 4 * m + u, :], base, float(wrow[u]), None,
                            op0=ALU.mult,
                        )

                nc.sync.dma_start(
                    out=yout[:, 64 * ts : 64 * (ts + T)], in_=ov
                )
                ts += T

    nc.compile()
    return nc


def _get_program(consts, tile_lens, act_groups):
    key = tuple(
        consts[k].tobytes() for k in ("A0", "B0", "AA1", "BB1", "w2", "w3")
    ) + (tuple(tile_lens), tuple(act_groups))
    if _cache.get(key) is None:
        _cache[key] = _build_program(consts, tile_lens, act_groups)
    return _cache[key]


def _run(x, W0, b0, W1, W2, W3, trace=False, tile_lens=None, act_groups=None):
    tile_lens = list(tile_lens or TILE_LENS)
    act_groups = tuple(act_groups or ACT_GROUPS)
    consts = _host_constants(
        np.asarray(W0, np.float32), np.asarray(b0, np.float32),
        np.asarray(W1, np.float32), np.asarray(W2, np.float32),
        np.asarray(W3, np.float32),
    )
    nc = _get_program(consts, tile_lens, act_groups)
    bf16 = ml_dtypes.bfloat16
    x = np.asarray(x, dtype=np.float32)
    # [M, P, 3, COLS] bf16, coordinate-major per core
    shards = np.ascontiguousarray(
        x.reshape(M_CORES, P, COLS, 3).transpose(0, 1, 3, 2)
    ).astype(bf16)
    in_maps = [{"xin": shards[c]} for c in range(M_CORES)]
    kwargs = {}
    if trace:
        kwargs = dict(trace=True, trace_cores=[0])
    res = run_bass_kernel_spmd(nc, in_maps, list(range(M_CORES)), **kwargs)
    out = np.empty((M_CORES, P, COLS, 64), dtype=np.float32)
    for c in range(M_CORES):
        arr = np.asarray(res.results[c]["yout"]).reshape(P, COLS * 64)
        ts = 0
        for T in tile_lens:
            out[c, :, ts : ts + T, :] = (
                arr[:, 64 * ts : 64 * (ts + T)]
                .reshape(P, 64, T)
                .transpose(0, 2, 1)
                .astype(np.float32)
            )
            ts += T
    return out.reshape(B, N, 16, 4), res


def kernel(x, W0, b0, W1, W2, W3):
    out, _ = _run(x, W0, b0, W1, W2, W3)
    return out


def kernel_traced(x, W0, b0, W1, W2, W3):
    """Like kernel(), but captures an NTFF profile; returns (out, results)."""
    import sys
    import types

    if "antenv.axon_hooks" not in sys.modules:
        mod = types.ModuleType("antenv.axon_hooks")
        _h = [None]
        mod.set_axon_ntff_profile_hook = lambda h: _h.__setitem__(0, h)
        mod.get_axon_ntff_profile_hook = lambda: _h[0]
        sys.modules["antenv.axon_hooks"] = mod
        if "/root/.axon_site" not in sys.path:
            sys.path.insert(0, "/root/.axon_site")
        from trn_agent_boot.trn_boot import _ntff_profile_via_ctypes

        mod.set_axon_ntff_profile_hook(
            _ntff_profile_via_ctypes("/opt/axon/libaxon_pjrt.so")
        )
    import concourse.bass_utils as bu

    bu.upload_artifacts = lambda tmpdir: "local://" + tmpdir
    return _run(x, W0, b0, W1, W2, W3, trace=True)
